# revision 11
# baseline (speedup 1.0000x reference)
"""Adaptive avg pool 2D (16,768,64,48) -> (16,768,7,7) on 8 TRN2 NeuronCores.

Data-parallel over B*C rows: 12288 rows of 64*48=3072 f32, 1536 rows/core.

Key idea vs the f32 baseline: DMA cost tracks the SBUF-side bytes and gpsimd
(SWDGE) DMAs can cast, so f32 DRAM -> f16 SBUF loads halve the per-tile
transfer (4369ns -> 2184ns), dropping the per-core DMA floor from ~52.4us
to ~26.2us. Inputs are N(0,1) so f16 keeps rel err ~6e-4 << 2e-2.

Schedule (per 128-row tile; engines balanced against the 2.2us DMA period):
  Loads are emitted up front so the Pool SEQ (which runs SWDGE descriptor
  gen, 994ns fixed + 0.34/desc) never blocks on a compute wait - the DMA
  stream stays gapless. Tiles 0..5 ride three DOUBLE loads (256 DRAM rows
  as [128, 2*3072]: two tiles side by side per partition), halving the gen
  count; their compute uses paired APs (leading [tile-pair] dim) to halve
  per-instruction overhead too.
  DVE: H pool (windows size 10 stride 9) as a pairwise tensor_tensor add
    tree (2x 16-bit mode, 0.52ns/elem; TensorReduce is 1 elem/cycle so
    reduces are kept small) + the q in {0,6} W reduce.
  Pool (gpsimd): q in 1..5 W pool as a pairwise add tree + the output
    scaling (tensor_scalar 1/70 | 1/80, f16->f32) for tiles 0..9, placed
    behind its own R2 so the in-order SEQs never wedge.
  Act: batched stores only (tiles 0-5, 6-9, 10-11) + tail-tile scaling.
  Tiles 10/11 are loaded in two chunks (rows 0..54 -> o<=5, 54..63 -> o=6):
  tile 10's W stage runs on Pool, tile 11's on DVE, so the post-stream tail
  is only the tiny o=6 slices plus one 98-column store.

Output DRAM layout is [128, 12*49] (tile-major columns); the host reorders
to [1536, 49]. A post-Tile pass legalizes multi-wait sync for this walrus
(max 1 wait/instruction, 2 on EventSemaphore).
  W windows (48->7): q=0:[0,7) q=6:[41,48) size 7; q=1..5 start 7q-1 size 8
  H windows (64->7): start 9*o, size 10 for all o
"""

import sys

_TRN_REPO = "/opt/trn_rl_repo"
if _TRN_REPO not in sys.path:
    sys.path.insert(0, _TRN_REPO)

import numpy as np

import concourse.bass as bass
import concourse.mybir as mybir
from concourse.tile import TileContext

B, C, H, W = 16, 768, 64, 48
HO, WO = 7, 7
NCORES = 8
ROWS = B * C // NCORES  # 1536 rows per core
P = 128
NTILES = ROWS // P  # 12
NPAIR = 3  # tiles 0..5 as three double loads
HA = 55  # split-tile chunk A rows 0..54 (covers o<=5); chunk B rows 54..63
TW = HO * W  # 336, tH elems per tile
TO = HO * WO  # 49, ot elems per tile

_nc_cache = None


def _legalize_multiwait(nc: bass.Bass) -> None:
    """Walrus (this version) accepts at most one sync wait per instruction
    (two for EventSemaphore). Tile's sem assignment can emit more (e.g. the
    kernel-tail drain waits on every DMA queue sem). Hoist all but the last
    wait into dedicated single-wait EventSemaphore carriers placed directly
    before the offending instruction on the same engine."""
    n = 0
    for b in nc.m.functions[0].blocks:
        insts = b.instructions
        i = 0
        while i < len(insts):
            inst = insts[i]
            si = inst.sync_info
            if si is not None and len(si.on_wait) > 1:
                waits = list(si.on_wait)
                carriers = []
                rest = waits[:-1]
                # EventSemaphore carriers can hold 2 waits each.
                for j in range(0, len(rest), 2):
                    n += 1
                    ev = mybir.InstEventSemaphore(
                        name=f"I-waitfix-{n}", ins=[], outs=[]
                    )
                    ev.engine = inst.engine
                    ev.sync_info = mybir.SyncInfo(
                        on_wait=rest[j : j + 2], on_update=[]
                    )
                    nc.register_instruction(ev)
                    carriers.append(ev)
                inst.sync_info = mybir.SyncInfo(
                    on_wait=[waits[-1]], on_update=list(si.on_update)
                )
                insts[i:i] = carriers
                i += len(carriers)
            i += 1


def _drop_const_memsets(nc: bass.Bass) -> None:
    """Remove the unconditional const-AP init memsets (Pool engine, emitted
    by Bass.__init__). This kernel never reads the const tiles (only Copy
    activations and immediate-scalar ops), and their Q7 launches sit on the
    critical lead-in path before the first SWDGE descriptor gen."""
    for b in nc.m.functions[0].blocks:
        b.instructions[:] = [
            inst
            for inst in b.instructions
            if not (
                isinstance(inst, mybir.InstMemset)
                and inst.outs
                and "const-" in getattr(inst.outs[0], "memref", "")
            )
        ]


def _build() -> bass.Bass:
    nc = bass.Bass()
    x = nc.dram_tensor("x", [ROWS, H * W], mybir.dt.float32, kind="ExternalInput")
    out = nc.dram_tensor(
        "out", [P, NTILES * TO], mybir.dt.float32, kind="ExternalOutput"
    )
    f16 = mybir.dt.float16
    X = mybir.AxisListType.X
    Copy = mybir.ActivationFunctionType.Copy
    HW = H * W

    def ap(tile, off, dims):
        return bass.AP(
            tensor=tile.tensor, offset=tile.offset + off,
            ap=[list(tile.ap[0])] + dims,
        )

    def pre_t(dims, s, np_):
        return ([[s, np_]] + dims) if np_ > 1 else dims

    with TileContext(nc) as tc:
        with (
            tc.tile_pool(name="xp", bufs=1) as xp,
            tc.tile_pool(name="yp", bufs=2) as yp,
            tc.tile_pool(name="wp", bufs=8) as wp,
            tc.tile_pool(name="hp", bufs=8) as hp,
            tc.tile_pool(name="op", bufs=8) as op,
            tc.tile_pool(name="sp", bufs=1) as sp,
        ):
            os_ = sp.tile([P, NTILES * TO], mybir.dt.float32)

            def h_adds(xt, tH, no, xoff, toff, np_=1, xstride=0, tstride=0):
                """H pool: tH[t, o, w] = sum_{j<10} xt rows 9o+j, pairwise
                tree on DVE (2x f16). np_ tiles per instruction (leading
                AP dim), xstride/tstride = per-tile strides."""
                y1 = yp.tile([P, 2 * HO * 5 * W], f16, tag="y1")
                y2 = yp.tile([P, 2 * HO * 2 * W], f16, tag="y2")
                nc.vector.tensor_add(
                    ap(y1, 0, pre_t([[5 * W, no], [W, 5], [1, W]], HO * 5 * W, np_)),
                    ap(xt, xoff, pre_t([[9 * W, no], [2 * W, 5], [1, W]], xstride, np_)),
                    ap(xt, xoff + W, pre_t([[9 * W, no], [2 * W, 5], [1, W]], xstride, np_)),
                )
                nc.vector.tensor_add(
                    ap(y2, 0, pre_t([[2 * W, no], [W, 2], [1, W]], HO * 2 * W, np_)),
                    ap(y1, 0, pre_t([[5 * W, no], [2 * W, 2], [1, W]], HO * 5 * W, np_)),
                    ap(y1, W, pre_t([[5 * W, no], [2 * W, 2], [1, W]], HO * 5 * W, np_)),
                )
                nc.vector.tensor_add(
                    ap(tH, toff, pre_t([[W, no], [1, W]], tstride, np_)),
                    ap(y2, 0, pre_t([[2 * W, no], [1, W]], HO * 2 * W, np_)),
                    ap(y2, W, pre_t([[2 * W, no], [1, W]], HO * 2 * W, np_)),
                )
                nc.vector.tensor_add(
                    ap(tH, toff, pre_t([[W, no], [1, W]], tstride, np_)),
                    ap(tH, toff, pre_t([[W, no], [1, W]], tstride, np_)),
                    ap(y1, 4 * W, pre_t([[5 * W, no], [1, W]], HO * 5 * W, np_)),
                )

            def w_r1_dve(tH, ot, no, toff, ooff, np_=1):
                # q in {0,6}: size-7 windows at w = 0 and 41 (DVE reduce, 1x).
                with nc.allow_low_precision(reason="f16 sums, x~N(0,1)"):
                    nc.vector.reduce_sum(
                        out=ap(ot, ooff, pre_t([[WO, no], [6, 2]], TO, np_)),
                        in_=ap(tH, toff, pre_t([[W, no], [41, 2], [1, 7]], TW, np_)),
                        axis=X,
                    )

            def w_r2_dve(tH, ot, no, toff, ooff):
                # q in 1..5: size-8 windows starting at 7q-1 (DVE reduce).
                with nc.allow_low_precision(reason="f16 sums, x~N(0,1)"):
                    nc.vector.reduce_sum(
                        out=ap(ot, ooff + 1, [[WO, no], [1, 5]]),
                        in_=ap(tH, toff + 6, [[W, no], [7, 5], [1, 8]]),
                        axis=X,
                    )

            def w_r2_pool(tH, ot, no, toff, ooff, np_=1):
                # q in 1..5 on gpsimd as a 3-instruction pairwise tree.
                w1 = wp.tile([P, 2 * HO * 5 * 4], f16, tag="w1")
                w2 = wp.tile([P, 2 * HO * 5 * 2], f16, tag="w2")
                nc.gpsimd.tensor_add(
                    ap(w1, 0, pre_t([[20, no], [4, 5], [1, 4]], 140, np_)),
                    ap(tH, toff + 6, pre_t([[W, no], [7, 5], [2, 4]], TW, np_)),
                    ap(tH, toff + 7, pre_t([[W, no], [7, 5], [2, 4]], TW, np_)),
                )
                nc.gpsimd.tensor_add(
                    ap(w2, 0, pre_t([[10, no], [2, 5], [1, 2]], 70, np_)),
                    ap(w1, 0, pre_t([[20, no], [4, 5], [2, 2]], 140, np_)),
                    ap(w1, 1, pre_t([[20, no], [4, 5], [2, 2]], 140, np_)),
                )
                nc.gpsimd.tensor_add(
                    ap(ot, ooff + 1, pre_t([[WO, no], [1, 5]], TO, np_)),
                    ap(w2, 0, pre_t([[10, no], [2, 5]], 70, np_)),
                    ap(w2, 1, pre_t([[10, no], [2, 5]], 70, np_)),
                )

            def w_r1_pool(tH, ot, no, toff, ooff):
                # q in {0,6} on gpsimd: pairwise over the 7-wide windows.
                v1 = wp.tile([P, HO * 2 * 3], f16, tag="v1")
                nc.gpsimd.tensor_add(
                    ap(v1, 0, [[6, no], [3, 2], [1, 3]]),
                    ap(tH, toff, [[W, no], [41, 2], [2, 3]]),
                    ap(tH, toff + 1, [[W, no], [41, 2], [2, 3]]),
                )
                nc.gpsimd.tensor_add(
                    ap(v1, 0, [[6, no], [3, 2]]),
                    ap(v1, 0, [[6, no], [3, 2]]),
                    ap(v1, 1, [[6, no], [3, 2]]),
                )
                nc.gpsimd.tensor_add(
                    ap(v1, 0, [[6, no], [3, 2]]),
                    ap(v1, 0, [[6, no], [3, 2]]),
                    ap(v1, 2, [[6, no], [3, 2]]),
                )
                nc.gpsimd.tensor_add(
                    ap(ot, ooff, [[WO, no], [6, 2]]),
                    ap(v1, 0, [[6, no], [3, 2]]),
                    ap(tH, toff + 6, [[W, no], [41, 2]]),
                )

            def muls_pool(ot, col, no, ooff, np_=1):
                # out = in / (10 * wsize_q) on gpsimd, f16 -> f32.
                nc.gpsimd.tensor_scalar_mul(
                    ap(os_, col, pre_t([[WO, no], [6, 2]], TO, np_)),
                    ap(ot, ooff, pre_t([[WO, no], [6, 2]], TO, np_)),
                    1.0 / 70.0,
                )
                nc.gpsimd.tensor_scalar_mul(
                    ap(os_, col + 1, pre_t([[WO, no], [1, 5]], TO, np_)),
                    ap(ot, ooff + 1, pre_t([[WO, no], [1, 5]], TO, np_)),
                    1.0 / 80.0,
                )

            def muls_act(ot, col, no, ooff):
                nc.scalar.activation(
                    ap(os_, col, [[WO, no], [6, 2]]),
                    ap(ot, ooff, [[WO, no], [6, 2]]),
                    Copy, scale=1.0 / 70.0,
                )
                nc.scalar.activation(
                    ap(os_, col + 1, [[WO, no], [1, 5]]),
                    ap(ot, ooff + 1, [[WO, no], [1, 5]]),
                    Copy, scale=1.0 / 80.0,
                )

            # --- all load chunks up front (gapless stream) ---
            HB = 36  # head-split boundary: o<=3 need rows<=36, o>=4 rows>=36
            xts = {}
            for k in range(NPAIR):  # double loads: tiles 2k, 2k+1
                xt = xp.tile([P, 2 * HW], f16, tag=f"xd{k}")
                if k == 0:
                    # Two H-chunks so DVE can start ~2.2us earlier on o<=3.
                    nc.gpsimd.dma_start(
                        out=ap(xt, 0, [[HW, 2], [1, (HB + 1) * W]]),
                        in_=bass.AP(
                            tensor=x, offset=0,
                            ap=[[HW, P], [P * HW, 2], [1, (HB + 1) * W]],
                        ),
                    )
                    nc.gpsimd.dma_start(
                        out=ap(xt, HB * W, [[HW, 2], [1, (H - HB) * W]]),
                        in_=bass.AP(
                            tensor=x, offset=HB * W,
                            ap=[[HW, P], [P * HW, 2], [1, (H - HB) * W]],
                        ),
                    )
                else:
                    nc.gpsimd.dma_start(
                        out=ap(xt, 0, [[HW, 2], [1, HW]]),
                        in_=bass.AP(
                            tensor=x, offset=2 * k * P * HW,
                            ap=[[HW, P], [P * HW, 2], [1, HW]],
                        ),
                    )
                xts[k] = xt
            for i in range(2 * NPAIR, NTILES - 2):  # singles 6..9
                xt = xp.tile([P, HW], f16, tag=f"xt{i}")
                nc.gpsimd.dma_start(out=xt, in_=x[i * P : (i + 1) * P, :])
                xts[i] = xt
            i10, i11 = NTILES - 2, NTILES - 1
            xabs = {}
            for i in (i10, i11):
                xa = xp.tile([P, HA * W], f16, tag=f"xa{i}")
                nc.gpsimd.dma_start(
                    out=xa, in_=x[i * P : (i + 1) * P, : HA * W]
                )
                xabs[i] = xa
            BW = (H - HA + 1) * W  # 480: B-chunk elems per tile
            xb = xp.tile([P, 2 * BW], f16, tag="xb")
            nc.gpsimd.dma_start(
                out=ap(xb, 0, [[BW, 2], [1, BW]]),
                in_=bass.AP(
                    tensor=x, offset=i10 * P * HW + (HA - 1) * W,
                    ap=[[HW, P], [P * HW, 2], [1, BW]],
                ),
            )
            xbs = {i10: (xb, 0), i11: (xb, BW)}

            # --- tiles 0..5 (halves of the three double loads) ---
            # walrus TENSOR3D encoding caps compute APs at 3 free dims, so
            # compute stays per-tile; only the loads (DMA descriptors) and
            # the muls (3 free dims) are pair-wide.
            for k in range(NPAIR):
                tH = hp.tile([P, 2 * TW], f16, tag="tHd")
                ot = op.tile([P, 2 * TO], f16, tag="otd")
                for c in range(2):
                    if k == 0:
                        h_adds(xts[k], tH, 4, c * HW, c * TW)
                        h_adds(xts[k], tH, 3, c * HW + 4 * 9 * W, c * TW + 4 * W)
                    else:
                        h_adds(xts[k], tH, HO, c * HW, c * TW)
                    w_r1_dve(tH, ot, HO, c * TW, c * TO)
                    w_r2_pool(tH, ot, HO, c * TW, c * TO)
                muls_pool(ot, 2 * k * TO, HO, 0, np_=2)

            # --- single tiles 6..9 ---
            for i in range(2 * NPAIR, NTILES - 2):
                tH = hp.tile([P, TW], f16, tag="tH")
                ot = op.tile([P, TO], f16, tag="ot")
                h_adds(xts[i], tH, HO, 0, 0)
                w_r1_dve(tH, ot, HO, 0, 0)
                w_r2_pool(tH, ot, HO, 0, 0)
                muls_pool(ot, i * TO, HO, 0)
                if i == 2 * NPAIR:
                    nc.scalar.dma_start(
                        out=out[:, 0 : 6 * TO],
                        in_=ap(os_, 0, [[1, 6 * TO]]),
                    )

            # --- tail tiles 10/11 ---
            tH10 = hp.tile([P, TW], f16, tag="tH10")
            ot10 = op.tile([P, TO], f16, tag="ot10")
            tH11 = hp.tile([P, TW], f16, tag="tH11")
            ot11 = op.tile([P, TO], f16, tag="ot11")
            c10, c11 = i10 * TO, i11 * TO
            nc.scalar.dma_start(
                out=out[:, 6 * TO : 10 * TO],
                in_=ap(os_, 6 * TO, [[1, 4 * TO]]),
            )
            h_adds(xabs[i10], tH10, 6, 0, 0)
            w_r1_pool(tH10, ot10, 6, 0, 0)
            w_r2_pool(tH10, ot10, 6, 0, 0)
            muls_pool(ot10, c10, 6, 0)
            h_adds(xabs[i11], tH11, 6, 0, 0)
            w_r1_dve(tH11, ot11, 6, 0, 0)
            w_r2_dve(tH11, ot11, 6, 0, 0)
            muls_act(ot11, c11, 6, 0)
            h_adds(xbs[i10][0], tH10, 1, xbs[i10][1], 6 * W)
            w_r1_pool(tH10, ot10, 1, 6 * W, 6 * WO)
            w_r2_pool(tH10, ot10, 1, 6 * W, 6 * WO)
            muls_pool(ot10, c10 + 6 * WO, 1, 6 * WO)
            h_adds(xbs[i11][0], tH11, 1, xbs[i11][1], 6 * W)
            w_r1_dve(tH11, ot11, 1, 6 * W, 6 * WO)
            w_r2_dve(tH11, ot11, 1, 6 * W, 6 * WO)
            muls_act(ot11, c11 + 6 * WO, 1, 6 * WO)
            nc.scalar.dma_start(
                out=out[:, 10 * TO :],
                in_=ap(os_, 10 * TO, [[1, 2 * TO]]),
            )
    _legalize_multiwait(nc)
    return nc


def kernel(x: np.ndarray) -> np.ndarray:
    global _nc_cache
    from concourse.bass_utils import run_bass_kernel_spmd

    xr = np.ascontiguousarray(np.asarray(x, dtype=np.float32).reshape(B * C, H * W))
    if _nc_cache is None:
        _nc_cache = _build()
    nc = _nc_cache
    in_maps = [
        {"x": xr[k * ROWS : (k + 1) * ROWS]} for k in range(NCORES)
    ]
    res = run_bass_kernel_spmd(nc, in_maps, list(range(NCORES)))
    # Per-core out is [128, NTILES*49] tile-major; reorder to [1536, 49].
    parts = [
        r["out"].reshape(P, NTILES, TO).transpose(1, 0, 2).reshape(ROWS, TO)
        for r in res.results
    ]
    return np.concatenate(parts, axis=0).reshape(B, C, HO, WO)


# revision 12
# speedup vs baseline: 1.1001x; 1.1001x over previous
"""Adaptive avg pool 2D (16,768,64,48) -> (16,768,7,7) on 8 TRN2 NeuronCores.

Data-parallel over B*C rows: 12288 rows of 64*48=3072 f32, 1536 rows/core.

Key idea vs the f32 baseline: DMA cost tracks the SBUF-side bytes and gpsimd
(SWDGE) DMAs can cast, so f32 DRAM -> f16 SBUF loads halve the per-tile
transfer (4369ns -> 2184ns), dropping the per-core DMA floor from ~52.4us
to ~26.2us. Inputs are N(0,1) so f16 keeps rel err ~6e-4 << 2e-2.

Schedule (per 128-row tile; engines balanced against the 2.2us DMA period):
  Loads are emitted up front so the Pool SEQ (which runs SWDGE descriptor
  gen, 994ns fixed + 0.34/desc) never blocks on a compute wait - the DMA
  stream stays gapless. Tiles 0..5 ride three DOUBLE loads (256 DRAM rows
  as [128, 2*3072]: two tiles side by side per partition), halving the gen
  count; their compute uses paired APs (leading [tile-pair] dim) to halve
  per-instruction overhead too.
  DVE: H pool (windows size 10 stride 9) as a pairwise tensor_tensor add
    tree (2x 16-bit mode, 0.52ns/elem; TensorReduce is 1 elem/cycle so
    reduces are kept small) + the q in {0,6} W reduce.
  Pool (gpsimd): q in 1..5 W pool as a pairwise add tree + the output
    scaling (tensor_scalar 1/70 | 1/80, f16->f32) for tiles 0..9, placed
    behind its own R2 so the in-order SEQs never wedge.
  Act: batched stores only (tiles 0-5, 6-9, 10-11) + tail-tile scaling.
  Tiles 10/11 are loaded in two chunks (rows 0..54 -> o<=5, 54..63 -> o=6):
  tile 10's W stage runs on Pool, tile 11's on DVE, so the post-stream tail
  is only the tiny o=6 slices plus one 98-column store.

Output DRAM layout is [128, 12*49] (tile-major columns); the host reorders
to [1536, 49]. A post-Tile pass legalizes multi-wait sync for this walrus
(max 1 wait/instruction, 2 on EventSemaphore).
  W windows (48->7): q=0:[0,7) q=6:[41,48) size 7; q=1..5 start 7q-1 size 8
  H windows (64->7): start 9*o, size 10 for all o
"""

import sys

_TRN_REPO = "/opt/trn_rl_repo"
if _TRN_REPO not in sys.path:
    sys.path.insert(0, _TRN_REPO)

import numpy as np

import concourse.bass as bass
import concourse.mybir as mybir
from concourse.tile import TileContext

B, C, H, W = 16, 768, 64, 48
HO, WO = 7, 7
NCORES = 8
ROWS = B * C // NCORES  # 1536 rows per core
P = 128
NTILES = ROWS // P  # 12
NPAIR = 3  # tiles 0..5 as three double loads
HA = 55  # split-tile chunk A rows 0..54 (covers o<=5); chunk B rows 54..63
TW = HO * W  # 336, tH elems per tile
TO = HO * WO  # 49, ot elems per tile

_nc_cache = None


def _legalize_multiwait(nc: bass.Bass) -> None:
    """Walrus (this version) accepts at most one sync wait per instruction
    (two for EventSemaphore). Tile's sem assignment can emit more (e.g. the
    kernel-tail drain waits on every DMA queue sem). Hoist all but the last
    wait into dedicated single-wait EventSemaphore carriers placed directly
    before the offending instruction on the same engine."""
    n = 0
    for b in nc.m.functions[0].blocks:
        insts = b.instructions
        i = 0
        while i < len(insts):
            inst = insts[i]
            si = inst.sync_info
            if si is not None and len(si.on_wait) > 1:
                waits = list(si.on_wait)
                carriers = []
                rest = waits[:-1]
                # EventSemaphore carriers can hold 2 waits each.
                for j in range(0, len(rest), 2):
                    n += 1
                    ev = mybir.InstEventSemaphore(
                        name=f"I-waitfix-{n}", ins=[], outs=[]
                    )
                    ev.engine = inst.engine
                    ev.sync_info = mybir.SyncInfo(
                        on_wait=rest[j : j + 2], on_update=[]
                    )
                    nc.register_instruction(ev)
                    carriers.append(ev)
                inst.sync_info = mybir.SyncInfo(
                    on_wait=[waits[-1]], on_update=list(si.on_update)
                )
                insts[i:i] = carriers
                i += len(carriers)
            i += 1


def _drop_const_memsets(nc: bass.Bass) -> None:
    """Remove the unconditional const-AP init memsets (Pool engine, emitted
    by Bass.__init__). This kernel never reads the const tiles (only Copy
    activations and immediate-scalar ops), and their Q7 launches sit on the
    critical lead-in path before the first SWDGE descriptor gen."""
    for b in nc.m.functions[0].blocks:
        b.instructions[:] = [
            inst
            for inst in b.instructions
            if not (
                isinstance(inst, mybir.InstMemset)
                and inst.outs
                and "const-" in getattr(inst.outs[0], "memref", "")
            )
        ]


def _build() -> bass.Bass:
    nc = bass.Bass()
    x = nc.dram_tensor("x", [ROWS, H * W], mybir.dt.float32, kind="ExternalInput")
    out = nc.dram_tensor(
        "out", [P, NTILES * TO], mybir.dt.float32, kind="ExternalOutput"
    )
    f16 = mybir.dt.float16
    X = mybir.AxisListType.X
    Copy = mybir.ActivationFunctionType.Copy
    HW = H * W

    def ap(tile, off, dims):
        return bass.AP(
            tensor=tile.tensor, offset=tile.offset + off,
            ap=[list(tile.ap[0])] + dims,
        )

    def pre_t(dims, s, np_):
        return ([[s, np_]] + dims) if np_ > 1 else dims

    with TileContext(nc) as tc:
        with (
            tc.tile_pool(name="xp", bufs=1) as xp,
            tc.tile_pool(name="yp", bufs=2) as yp,
            tc.tile_pool(name="wp", bufs=8) as wp,
            tc.tile_pool(name="hp", bufs=8) as hp,
            tc.tile_pool(name="op", bufs=8) as op,
            tc.tile_pool(name="sp", bufs=1) as sp,
        ):
            os_ = sp.tile([P, NTILES * TO], mybir.dt.float32)

            def h_adds(xt, tH, no, xoff, toff, np_=1, xstride=0, tstride=0):
                """H pool: tH[t, o, w] = sum_{j<10} xt rows 9o+j, pairwise
                tree on DVE (2x f16). np_ tiles per instruction (leading
                AP dim), xstride/tstride = per-tile strides."""
                y1 = yp.tile([P, 2 * HO * 5 * W], f16, tag="y1")
                y2 = yp.tile([P, 2 * HO * 2 * W], f16, tag="y2")
                nc.vector.tensor_add(
                    ap(y1, 0, pre_t([[5 * W, no], [W, 5], [1, W]], HO * 5 * W, np_)),
                    ap(xt, xoff, pre_t([[9 * W, no], [2 * W, 5], [1, W]], xstride, np_)),
                    ap(xt, xoff + W, pre_t([[9 * W, no], [2 * W, 5], [1, W]], xstride, np_)),
                )
                nc.vector.tensor_add(
                    ap(y2, 0, pre_t([[2 * W, no], [W, 2], [1, W]], HO * 2 * W, np_)),
                    ap(y1, 0, pre_t([[5 * W, no], [2 * W, 2], [1, W]], HO * 5 * W, np_)),
                    ap(y1, W, pre_t([[5 * W, no], [2 * W, 2], [1, W]], HO * 5 * W, np_)),
                )
                nc.vector.tensor_add(
                    ap(tH, toff, pre_t([[W, no], [1, W]], tstride, np_)),
                    ap(y2, 0, pre_t([[2 * W, no], [1, W]], HO * 2 * W, np_)),
                    ap(y2, W, pre_t([[2 * W, no], [1, W]], HO * 2 * W, np_)),
                )
                nc.vector.tensor_add(
                    ap(tH, toff, pre_t([[W, no], [1, W]], tstride, np_)),
                    ap(tH, toff, pre_t([[W, no], [1, W]], tstride, np_)),
                    ap(y1, 4 * W, pre_t([[5 * W, no], [1, W]], HO * 5 * W, np_)),
                )

            def w_r1_dve(tH, ot, no, toff, ooff, np_=1):
                # q in {0,6}: size-7 windows at w = 0 and 41 (DVE reduce, 1x).
                with nc.allow_low_precision(reason="f16 sums, x~N(0,1)"):
                    nc.vector.reduce_sum(
                        out=ap(ot, ooff, pre_t([[WO, no], [6, 2]], TO, np_)),
                        in_=ap(tH, toff, pre_t([[W, no], [41, 2], [1, 7]], TW, np_)),
                        axis=X,
                    )

            def w_r2_dve(tH, ot, no, toff, ooff):
                # q in 1..5: size-8 windows starting at 7q-1 (DVE reduce).
                with nc.allow_low_precision(reason="f16 sums, x~N(0,1)"):
                    nc.vector.reduce_sum(
                        out=ap(ot, ooff + 1, [[WO, no], [1, 5]]),
                        in_=ap(tH, toff + 6, [[W, no], [7, 5], [1, 8]]),
                        axis=X,
                    )

            def w_r2_pool(tH, ot, no, toff, ooff, np_=1):
                # q in 1..5 on gpsimd as a 3-instruction pairwise tree.
                w1 = wp.tile([P, 2 * HO * 5 * 4], f16, tag="w1")
                w2 = wp.tile([P, 2 * HO * 5 * 2], f16, tag="w2")
                nc.gpsimd.tensor_add(
                    ap(w1, 0, pre_t([[20, no], [4, 5], [1, 4]], 140, np_)),
                    ap(tH, toff + 6, pre_t([[W, no], [7, 5], [2, 4]], TW, np_)),
                    ap(tH, toff + 7, pre_t([[W, no], [7, 5], [2, 4]], TW, np_)),
                )
                nc.gpsimd.tensor_add(
                    ap(w2, 0, pre_t([[10, no], [2, 5], [1, 2]], 70, np_)),
                    ap(w1, 0, pre_t([[20, no], [4, 5], [2, 2]], 140, np_)),
                    ap(w1, 1, pre_t([[20, no], [4, 5], [2, 2]], 140, np_)),
                )
                nc.gpsimd.tensor_add(
                    ap(ot, ooff + 1, pre_t([[WO, no], [1, 5]], TO, np_)),
                    ap(w2, 0, pre_t([[10, no], [2, 5]], 70, np_)),
                    ap(w2, 1, pre_t([[10, no], [2, 5]], 70, np_)),
                )

            def w_r1_pool(tH, ot, no, toff, ooff):
                # q in {0,6} on gpsimd: pairwise over the 7-wide windows.
                v1 = wp.tile([P, HO * 2 * 3], f16, tag="v1")
                nc.gpsimd.tensor_add(
                    ap(v1, 0, [[6, no], [3, 2], [1, 3]]),
                    ap(tH, toff, [[W, no], [41, 2], [2, 3]]),
                    ap(tH, toff + 1, [[W, no], [41, 2], [2, 3]]),
                )
                nc.gpsimd.tensor_add(
                    ap(v1, 0, [[6, no], [3, 2]]),
                    ap(v1, 0, [[6, no], [3, 2]]),
                    ap(v1, 1, [[6, no], [3, 2]]),
                )
                nc.gpsimd.tensor_add(
                    ap(v1, 0, [[6, no], [3, 2]]),
                    ap(v1, 0, [[6, no], [3, 2]]),
                    ap(v1, 2, [[6, no], [3, 2]]),
                )
                nc.gpsimd.tensor_add(
                    ap(ot, ooff, [[WO, no], [6, 2]]),
                    ap(v1, 0, [[6, no], [3, 2]]),
                    ap(tH, toff + 6, [[W, no], [41, 2]]),
                )

            def muls_pool(ot, col, no, ooff, np_=1):
                # out = in / (10 * wsize_q) on gpsimd, f16 -> f32.
                nc.gpsimd.tensor_scalar_mul(
                    ap(os_, col, pre_t([[WO, no], [6, 2]], TO, np_)),
                    ap(ot, ooff, pre_t([[WO, no], [6, 2]], TO, np_)),
                    1.0 / 70.0,
                )
                nc.gpsimd.tensor_scalar_mul(
                    ap(os_, col + 1, pre_t([[WO, no], [1, 5]], TO, np_)),
                    ap(ot, ooff + 1, pre_t([[WO, no], [1, 5]], TO, np_)),
                    1.0 / 80.0,
                )

            def muls_act(ot, col, no, ooff):
                nc.scalar.activation(
                    ap(os_, col, [[WO, no], [6, 2]]),
                    ap(ot, ooff, [[WO, no], [6, 2]]),
                    Copy, scale=1.0 / 70.0,
                )
                nc.scalar.activation(
                    ap(os_, col + 1, [[WO, no], [1, 5]]),
                    ap(ot, ooff + 1, [[WO, no], [1, 5]]),
                    Copy, scale=1.0 / 80.0,
                )

            # --- all load chunks up front (gapless stream) ---
            HB = 36  # head-split boundary: o<=3 need rows<=36, o>=4 rows>=36
            xts = {}
            for k in range(NPAIR):  # double loads: tiles 2k, 2k+1
                if k == 0:
                    # Two H-chunks in separate tiles (a shared tile would
                    # put a WAW sem between the DMAs, serializing the gens)
                    # so DVE can start ~2.2us earlier on o<=3.
                    na, nb = (HB + 1) * W, (H - HB) * W
                    xta = xp.tile([P, 2 * na], f16, tag="xd0a")
                    xtb = xp.tile([P, 2 * nb], f16, tag="xd0b")
                    nc.gpsimd.dma_start(
                        out=ap(xta, 0, [[na, 2], [1, na]]),
                        in_=bass.AP(
                            tensor=x, offset=0,
                            ap=[[HW, P], [P * HW, 2], [1, na]],
                        ),
                    )
                    nc.gpsimd.dma_start(
                        out=ap(xtb, 0, [[nb, 2], [1, nb]]),
                        in_=bass.AP(
                            tensor=x, offset=HB * W,
                            ap=[[HW, P], [P * HW, 2], [1, nb]],
                        ),
                    )
                    xts[k] = (xta, xtb)
                else:
                    xt = xp.tile([P, 2 * HW], f16, tag=f"xd{k}")
                    nc.gpsimd.dma_start(
                        out=ap(xt, 0, [[HW, 2], [1, HW]]),
                        in_=bass.AP(
                            tensor=x, offset=2 * k * P * HW,
                            ap=[[HW, P], [P * HW, 2], [1, HW]],
                        ),
                    )
                    xts[k] = xt
            for i in range(2 * NPAIR, NTILES - 2):  # singles 6..9
                xt = xp.tile([P, HW], f16, tag=f"xt{i}")
                nc.gpsimd.dma_start(out=xt, in_=x[i * P : (i + 1) * P, :])
                xts[i] = xt
            i10, i11 = NTILES - 2, NTILES - 1
            xabs = {}
            for i in (i10, i11):
                xa = xp.tile([P, HA * W], f16, tag=f"xa{i}")
                nc.gpsimd.dma_start(
                    out=xa, in_=x[i * P : (i + 1) * P, : HA * W]
                )
                xabs[i] = xa
            BW = (H - HA + 1) * W  # 480: B-chunk elems per tile
            xb = xp.tile([P, 2 * BW], f16, tag="xb")
            nc.gpsimd.dma_start(
                out=ap(xb, 0, [[BW, 2], [1, BW]]),
                in_=bass.AP(
                    tensor=x, offset=i10 * P * HW + (HA - 1) * W,
                    ap=[[HW, P], [P * HW, 2], [1, BW]],
                ),
            )
            xbs = {i10: (xb, 0), i11: (xb, BW)}

            # --- tiles 0..5 (halves of the three double loads) ---
            # walrus TENSOR3D encoding caps compute APs at 3 free dims, so
            # compute stays per-tile; only the loads (DMA descriptors) and
            # the muls (3 free dims) are pair-wide.
            for k in range(NPAIR):
                tH = hp.tile([P, 2 * TW], f16, tag="tHd")
                ot = op.tile([P, 2 * TO], f16, tag="otd")
                for c in range(2):
                    if k == 0:
                        xta, xtb = xts[k]
                        h_adds(xta, tH, 4, c * (HB + 1) * W, c * TW)
                        h_adds(xtb, tH, 3, c * (H - HB) * W, c * TW + 4 * W)
                    else:
                        h_adds(xts[k], tH, HO, c * HW, c * TW)
                    w_r1_dve(tH, ot, HO, c * TW, c * TO)
                    w_r2_pool(tH, ot, HO, c * TW, c * TO)
                muls_pool(ot, 2 * k * TO, HO, 0, np_=2)

            # --- single tiles 6..9 ---
            for i in range(2 * NPAIR, NTILES - 2):
                tH = hp.tile([P, TW], f16, tag="tH")
                ot = op.tile([P, TO], f16, tag="ot")
                h_adds(xts[i], tH, HO, 0, 0)
                w_r1_dve(tH, ot, HO, 0, 0)
                w_r2_pool(tH, ot, HO, 0, 0)
                muls_pool(ot, i * TO, HO, 0)
                if i == 2 * NPAIR:
                    nc.scalar.dma_start(
                        out=out[:, 0 : 6 * TO],
                        in_=ap(os_, 0, [[1, 6 * TO]]),
                    )

            # --- tail tiles 10/11 ---
            tH10 = hp.tile([P, TW], f16, tag="tH10")
            ot10 = op.tile([P, TO], f16, tag="ot10")
            tH11 = hp.tile([P, TW], f16, tag="tH11")
            ot11 = op.tile([P, TO], f16, tag="ot11")
            c10, c11 = i10 * TO, i11 * TO
            nc.scalar.dma_start(
                out=out[:, 6 * TO : 10 * TO],
                in_=ap(os_, 6 * TO, [[1, 4 * TO]]),
            )
            h_adds(xabs[i10], tH10, 6, 0, 0)
            w_r1_pool(tH10, ot10, 6, 0, 0)
            w_r2_pool(tH10, ot10, 6, 0, 0)
            muls_pool(ot10, c10, 6, 0)
            h_adds(xabs[i11], tH11, 6, 0, 0)
            w_r1_dve(tH11, ot11, 6, 0, 0)
            w_r2_dve(tH11, ot11, 6, 0, 0)
            muls_act(ot11, c11, 6, 0)
            h_adds(xbs[i10][0], tH10, 1, xbs[i10][1], 6 * W)
            w_r1_pool(tH10, ot10, 1, 6 * W, 6 * WO)
            w_r2_pool(tH10, ot10, 1, 6 * W, 6 * WO)
            muls_pool(ot10, c10 + 6 * WO, 1, 6 * WO)
            h_adds(xbs[i11][0], tH11, 1, xbs[i11][1], 6 * W)
            w_r1_dve(tH11, ot11, 1, 6 * W, 6 * WO)
            w_r2_dve(tH11, ot11, 1, 6 * W, 6 * WO)
            muls_act(ot11, c11 + 6 * WO, 1, 6 * WO)
            nc.scalar.dma_start(
                out=out[:, 10 * TO :],
                in_=ap(os_, 10 * TO, [[1, 2 * TO]]),
            )
    _legalize_multiwait(nc)
    return nc


def kernel(x: np.ndarray) -> np.ndarray:
    global _nc_cache
    from concourse.bass_utils import run_bass_kernel_spmd

    xr = np.ascontiguousarray(np.asarray(x, dtype=np.float32).reshape(B * C, H * W))
    if _nc_cache is None:
        _nc_cache = _build()
    nc = _nc_cache
    in_maps = [
        {"x": xr[k * ROWS : (k + 1) * ROWS]} for k in range(NCORES)
    ]
    res = run_bass_kernel_spmd(nc, in_maps, list(range(NCORES)))
    # Per-core out is [128, NTILES*49] tile-major; reorder to [1536, 49].
    parts = [
        r["out"].reshape(P, NTILES, TO).transpose(1, 0, 2).reshape(ROWS, TO)
        for r in res.results
    ]
    return np.concatenate(parts, axis=0).reshape(B, C, HO, WO)


# revision 13
# speedup vs baseline: 1.1086x; 1.0078x over previous
"""Adaptive avg pool 2D (16,768,64,48) -> (16,768,7,7) on 8 TRN2 NeuronCores.

Data-parallel over B*C rows: 12288 rows of 64*48=3072 f32, 1536 rows/core.

Key idea vs the f32 baseline: DMA cost tracks the SBUF-side bytes and gpsimd
(SWDGE) DMAs can cast, so f32 DRAM -> f16 SBUF loads halve the per-tile
transfer (4369ns -> 2184ns), dropping the per-core DMA floor from ~52.4us
to ~26.2us. Inputs are N(0,1) so f16 keeps rel err ~6e-4 << 2e-2.

Schedule (per 128-row tile; engines balanced against the 2.2us DMA period):
  Loads are emitted up front so the Pool SEQ (which runs SWDGE descriptor
  gen, 994ns fixed + 0.34/desc) never blocks on a compute wait - the DMA
  stream stays gapless. Tiles 0..5 ride three DOUBLE loads (256 DRAM rows
  as [128, 2*3072]: two tiles side by side per partition), halving the gen
  count; their compute uses paired APs (leading [tile-pair] dim) to halve
  per-instruction overhead too.
  DVE: H pool (windows size 10 stride 9) as a pairwise tensor_tensor add
    tree (2x 16-bit mode, 0.52ns/elem; TensorReduce is 1 elem/cycle so
    reduces are kept small) + the q in {0,6} W reduce.
  Pool (gpsimd): q in 1..5 W pool as a pairwise add tree + the output
    scaling (tensor_scalar 1/70 | 1/80, f16->f32) for tiles 0..9, placed
    behind its own R2 so the in-order SEQs never wedge.
  Act: batched stores only (tiles 0-5, 6-9, 10-11) + tail-tile scaling.
  Tiles 10/11 are loaded in two chunks (rows 0..54 -> o<=5, 54..63 -> o=6):
  tile 10's W stage runs on Pool, tile 11's on DVE, so the post-stream tail
  is only the tiny o=6 slices plus one 98-column store.

Output DRAM layout is [128, 12*49] (tile-major columns); the host reorders
to [1536, 49]. A post-Tile pass legalizes multi-wait sync for this walrus
(max 1 wait/instruction, 2 on EventSemaphore).
  W windows (48->7): q=0:[0,7) q=6:[41,48) size 7; q=1..5 start 7q-1 size 8
  H windows (64->7): start 9*o, size 10 for all o
"""

import sys

_TRN_REPO = "/opt/trn_rl_repo"
if _TRN_REPO not in sys.path:
    sys.path.insert(0, _TRN_REPO)

import numpy as np

import concourse.bass as bass
import concourse.mybir as mybir
from concourse.tile import TileContext

B, C, H, W = 16, 768, 64, 48
HO, WO = 7, 7
NCORES = 8
ROWS = B * C // NCORES  # 1536 rows per core
P = 128
NTILES = ROWS // P  # 12
NPAIR = 3  # tiles 0..5 as three double loads
HA = 55  # split-tile chunk A rows 0..54 (covers o<=5); chunk B rows 54..63
TW = HO * W  # 336, tH elems per tile
TO = HO * WO  # 49, ot elems per tile

_nc_cache = None


def _legalize_multiwait(nc: bass.Bass) -> None:
    """Walrus (this version) accepts at most one sync wait per instruction
    (two for EventSemaphore). Tile's sem assignment can emit more (e.g. the
    kernel-tail drain waits on every DMA queue sem). Hoist all but the last
    wait into dedicated single-wait EventSemaphore carriers placed directly
    before the offending instruction on the same engine."""
    n = 0
    for b in nc.m.functions[0].blocks:
        insts = b.instructions
        i = 0
        while i < len(insts):
            inst = insts[i]
            si = inst.sync_info
            if si is not None and len(si.on_wait) > 1:
                waits = list(si.on_wait)
                carriers = []
                rest = waits[:-1]
                # EventSemaphore carriers can hold 2 waits each.
                for j in range(0, len(rest), 2):
                    n += 1
                    ev = mybir.InstEventSemaphore(
                        name=f"I-waitfix-{n}", ins=[], outs=[]
                    )
                    ev.engine = inst.engine
                    ev.sync_info = mybir.SyncInfo(
                        on_wait=rest[j : j + 2], on_update=[]
                    )
                    nc.register_instruction(ev)
                    carriers.append(ev)
                inst.sync_info = mybir.SyncInfo(
                    on_wait=[waits[-1]], on_update=list(si.on_update)
                )
                insts[i:i] = carriers
                i += len(carriers)
            i += 1


def _drop_const_memsets(nc: bass.Bass) -> None:
    """Remove the unconditional const-AP init memsets (Pool engine, emitted
    by Bass.__init__). This kernel never reads the const tiles (only Copy
    activations and immediate-scalar ops), and their Q7 launches sit on the
    critical lead-in path before the first SWDGE descriptor gen."""
    for b in nc.m.functions[0].blocks:
        b.instructions[:] = [
            inst
            for inst in b.instructions
            if not (
                isinstance(inst, mybir.InstMemset)
                and inst.outs
                and "const-" in getattr(inst.outs[0], "memref", "")
            )
        ]


def _build() -> bass.Bass:
    nc = bass.Bass()
    x = nc.dram_tensor("x", [ROWS, H * W], mybir.dt.float32, kind="ExternalInput")
    out = nc.dram_tensor(
        "out", [P, NTILES * TO], mybir.dt.float32, kind="ExternalOutput"
    )
    f16 = mybir.dt.float16
    X = mybir.AxisListType.X
    Copy = mybir.ActivationFunctionType.Copy
    HW = H * W

    def ap(tile, off, dims):
        return bass.AP(
            tensor=tile.tensor, offset=tile.offset + off,
            ap=[list(tile.ap[0])] + dims,
        )

    def pre_t(dims, s, np_):
        return ([[s, np_]] + dims) if np_ > 1 else dims

    with TileContext(nc) as tc:
        with (
            tc.tile_pool(name="xp", bufs=1) as xp,
            tc.tile_pool(name="yp", bufs=2) as yp,
            tc.tile_pool(name="wp", bufs=8) as wp,
            tc.tile_pool(name="hp", bufs=8) as hp,
            tc.tile_pool(name="op", bufs=8) as op,
            tc.tile_pool(name="sp", bufs=1) as sp,
        ):
            os_ = sp.tile([P, NTILES * TO], mybir.dt.float32)

            def h_adds(xt, tH, no, xoff, toff, np_=1, xstride=0, tstride=0):
                """H pool: tH[t, o, w] = sum_{j<10} xt rows 9o+j, pairwise
                tree on DVE (2x f16). np_ tiles per instruction (leading
                AP dim), xstride/tstride = per-tile strides."""
                y1 = yp.tile([P, 2 * HO * 5 * W], f16, tag="y1")
                y2 = yp.tile([P, 2 * HO * 2 * W], f16, tag="y2")
                nc.vector.tensor_add(
                    ap(y1, 0, pre_t([[5 * W, no], [W, 5], [1, W]], HO * 5 * W, np_)),
                    ap(xt, xoff, pre_t([[9 * W, no], [2 * W, 5], [1, W]], xstride, np_)),
                    ap(xt, xoff + W, pre_t([[9 * W, no], [2 * W, 5], [1, W]], xstride, np_)),
                )
                nc.vector.tensor_add(
                    ap(y2, 0, pre_t([[2 * W, no], [W, 2], [1, W]], HO * 2 * W, np_)),
                    ap(y1, 0, pre_t([[5 * W, no], [2 * W, 2], [1, W]], HO * 5 * W, np_)),
                    ap(y1, W, pre_t([[5 * W, no], [2 * W, 2], [1, W]], HO * 5 * W, np_)),
                )
                nc.vector.tensor_add(
                    ap(tH, toff, pre_t([[W, no], [1, W]], tstride, np_)),
                    ap(y2, 0, pre_t([[2 * W, no], [1, W]], HO * 2 * W, np_)),
                    ap(y2, W, pre_t([[2 * W, no], [1, W]], HO * 2 * W, np_)),
                )
                nc.vector.tensor_add(
                    ap(tH, toff, pre_t([[W, no], [1, W]], tstride, np_)),
                    ap(tH, toff, pre_t([[W, no], [1, W]], tstride, np_)),
                    ap(y1, 4 * W, pre_t([[5 * W, no], [1, W]], HO * 5 * W, np_)),
                )

            def w_r1_dve(tH, ot, no, toff, ooff, np_=1):
                # q in {0,6}: size-7 windows at w = 0 and 41 (DVE reduce, 1x).
                with nc.allow_low_precision(reason="f16 sums, x~N(0,1)"):
                    nc.vector.reduce_sum(
                        out=ap(ot, ooff, pre_t([[WO, no], [6, 2]], TO, np_)),
                        in_=ap(tH, toff, pre_t([[W, no], [41, 2], [1, 7]], TW, np_)),
                        axis=X,
                    )

            def w_r2_dve(tH, ot, no, toff, ooff):
                # q in 1..5: size-8 windows starting at 7q-1 (DVE reduce).
                with nc.allow_low_precision(reason="f16 sums, x~N(0,1)"):
                    nc.vector.reduce_sum(
                        out=ap(ot, ooff + 1, [[WO, no], [1, 5]]),
                        in_=ap(tH, toff + 6, [[W, no], [7, 5], [1, 8]]),
                        axis=X,
                    )

            def w_r2_pool(tH, ot, no, toff, ooff, np_=1):
                # q in 1..5 on gpsimd as a 3-instruction pairwise tree.
                w1 = wp.tile([P, 2 * HO * 5 * 4], f16, tag="w1")
                w2 = wp.tile([P, 2 * HO * 5 * 2], f16, tag="w2")
                nc.gpsimd.tensor_add(
                    ap(w1, 0, pre_t([[20, no], [4, 5], [1, 4]], 140, np_)),
                    ap(tH, toff + 6, pre_t([[W, no], [7, 5], [2, 4]], TW, np_)),
                    ap(tH, toff + 7, pre_t([[W, no], [7, 5], [2, 4]], TW, np_)),
                )
                nc.gpsimd.tensor_add(
                    ap(w2, 0, pre_t([[10, no], [2, 5], [1, 2]], 70, np_)),
                    ap(w1, 0, pre_t([[20, no], [4, 5], [2, 2]], 140, np_)),
                    ap(w1, 1, pre_t([[20, no], [4, 5], [2, 2]], 140, np_)),
                )
                nc.gpsimd.tensor_add(
                    ap(ot, ooff + 1, pre_t([[WO, no], [1, 5]], TO, np_)),
                    ap(w2, 0, pre_t([[10, no], [2, 5]], 70, np_)),
                    ap(w2, 1, pre_t([[10, no], [2, 5]], 70, np_)),
                )

            def w_r1_pool(tH, ot, no, toff, ooff):
                # q in {0,6} on gpsimd: pairwise over the 7-wide windows.
                v1 = wp.tile([P, HO * 2 * 3], f16, tag="v1")
                nc.gpsimd.tensor_add(
                    ap(v1, 0, [[6, no], [3, 2], [1, 3]]),
                    ap(tH, toff, [[W, no], [41, 2], [2, 3]]),
                    ap(tH, toff + 1, [[W, no], [41, 2], [2, 3]]),
                )
                nc.gpsimd.tensor_add(
                    ap(v1, 0, [[6, no], [3, 2]]),
                    ap(v1, 0, [[6, no], [3, 2]]),
                    ap(v1, 1, [[6, no], [3, 2]]),
                )
                nc.gpsimd.tensor_add(
                    ap(v1, 0, [[6, no], [3, 2]]),
                    ap(v1, 0, [[6, no], [3, 2]]),
                    ap(v1, 2, [[6, no], [3, 2]]),
                )
                nc.gpsimd.tensor_add(
                    ap(ot, ooff, [[WO, no], [6, 2]]),
                    ap(v1, 0, [[6, no], [3, 2]]),
                    ap(tH, toff + 6, [[W, no], [41, 2]]),
                )

            def muls_pool(ot, col, no, ooff, np_=1):
                # out = in / (10 * wsize_q) on gpsimd, f16 -> f32.
                nc.gpsimd.tensor_scalar_mul(
                    ap(os_, col, pre_t([[WO, no], [6, 2]], TO, np_)),
                    ap(ot, ooff, pre_t([[WO, no], [6, 2]], TO, np_)),
                    1.0 / 70.0,
                )
                nc.gpsimd.tensor_scalar_mul(
                    ap(os_, col + 1, pre_t([[WO, no], [1, 5]], TO, np_)),
                    ap(ot, ooff + 1, pre_t([[WO, no], [1, 5]], TO, np_)),
                    1.0 / 80.0,
                )

            def muls_act(ot, col, no, ooff):
                nc.scalar.activation(
                    ap(os_, col, [[WO, no], [6, 2]]),
                    ap(ot, ooff, [[WO, no], [6, 2]]),
                    Copy, scale=1.0 / 70.0,
                )
                nc.scalar.activation(
                    ap(os_, col + 1, [[WO, no], [1, 5]]),
                    ap(ot, ooff + 1, [[WO, no], [1, 5]]),
                    Copy, scale=1.0 / 80.0,
                )

            # --- all load chunks up front (gapless stream) ---
            xts = {}
            for i in range(NTILES - 2):
                xt = xp.tile([P, HW], f16, tag=f"xt{i}")
                nc.gpsimd.dma_start(out=xt, in_=x[i * P : (i + 1) * P, :])
                xts[i] = xt
            i10, i11 = NTILES - 2, NTILES - 1
            xabs = {}
            for i in (i10, i11):
                xa = xp.tile([P, HA * W], f16, tag=f"xa{i}")
                nc.gpsimd.dma_start(
                    out=xa, in_=x[i * P : (i + 1) * P, : HA * W]
                )
                xabs[i] = xa
            BW = (H - HA + 1) * W  # 480: B-chunk elems per tile
            xb = xp.tile([P, 2 * BW], f16, tag="xb")
            nc.gpsimd.dma_start(
                out=ap(xb, 0, [[BW, 2], [1, BW]]),
                in_=bass.AP(
                    tensor=x, offset=i10 * P * HW + (HA - 1) * W,
                    ap=[[HW, P], [P * HW, 2], [1, BW]],
                ),
            )
            xbs = {i10: (xb, 0), i11: (xb, BW)}

            # --- steady tiles 0..9 ---
            for i in range(NTILES - 2):
                tH = hp.tile([P, TW], f16, tag="tH")
                ot = op.tile([P, TO], f16, tag="ot")
                h_adds(xts[i], tH, HO, 0, 0)
                w_r1_dve(tH, ot, HO, 0, 0)
                w_r2_pool(tH, ot, HO, 0, 0)
                muls_pool(ot, i * TO, HO, 0)
                if i == 6:
                    nc.scalar.dma_start(
                        out=out[:, 0 : 6 * TO],
                        in_=ap(os_, 0, [[1, 6 * TO]]),
                    )

            # --- tail tiles 10/11 ---
            tH10 = hp.tile([P, TW], f16, tag="tH10")
            ot10 = op.tile([P, TO], f16, tag="ot10")
            tH11 = hp.tile([P, TW], f16, tag="tH11")
            ot11 = op.tile([P, TO], f16, tag="ot11")
            c10, c11 = i10 * TO, i11 * TO
            nc.scalar.dma_start(
                out=out[:, 6 * TO : 10 * TO],
                in_=ap(os_, 6 * TO, [[1, 4 * TO]]),
            )
            h_adds(xabs[i10], tH10, 6, 0, 0)
            w_r1_pool(tH10, ot10, 6, 0, 0)
            w_r2_pool(tH10, ot10, 6, 0, 0)
            muls_pool(ot10, c10, 6, 0)
            h_adds(xabs[i11], tH11, 6, 0, 0)
            w_r1_dve(tH11, ot11, 6, 0, 0)
            w_r2_dve(tH11, ot11, 6, 0, 0)
            muls_act(ot11, c11, 6, 0)
            h_adds(xbs[i10][0], tH10, 1, xbs[i10][1], 6 * W)
            w_r1_pool(tH10, ot10, 1, 6 * W, 6 * WO)
            w_r2_pool(tH10, ot10, 1, 6 * W, 6 * WO)
            muls_pool(ot10, c10 + 6 * WO, 1, 6 * WO)
            h_adds(xbs[i11][0], tH11, 1, xbs[i11][1], 6 * W)
            w_r1_dve(tH11, ot11, 1, 6 * W, 6 * WO)
            w_r2_dve(tH11, ot11, 1, 6 * W, 6 * WO)
            muls_act(ot11, c11 + 6 * WO, 1, 6 * WO)
            nc.scalar.dma_start(
                out=out[:, 10 * TO :],
                in_=ap(os_, 10 * TO, [[1, 2 * TO]]),
            )
    _legalize_multiwait(nc)
    return nc


def kernel(x: np.ndarray) -> np.ndarray:
    global _nc_cache
    from concourse.bass_utils import run_bass_kernel_spmd

    xr = np.ascontiguousarray(np.asarray(x, dtype=np.float32).reshape(B * C, H * W))
    if _nc_cache is None:
        _nc_cache = _build()
    nc = _nc_cache
    in_maps = [
        {"x": xr[k * ROWS : (k + 1) * ROWS]} for k in range(NCORES)
    ]
    res = run_bass_kernel_spmd(nc, in_maps, list(range(NCORES)))
    # Per-core out is [128, NTILES*49] tile-major; reorder to [1536, 49].
    parts = [
        r["out"].reshape(P, NTILES, TO).transpose(1, 0, 2).reshape(ROWS, TO)
        for r in res.results
    ]
    return np.concatenate(parts, axis=0).reshape(B, C, HO, WO)
